# revision 8
# baseline (speedup 1.0000x reference)
"""Trainium2 Bass kernel v2 for the TimeSformer-style divided space-time block.

Data-parallel over B (8 cores). Per core, the residual stream lives in SBUF
feature-major as bf16 for the whole block:

  - x loaded once via DMA-transpose (host pre-casts x to bf16, pads to 1664
    rows); no DRAM round trips between the three stages.
  - LayerNorm is computed feature-major: Sum(x) / Sum(x^2) via ones-matmuls on
    the PE (bf16), ACT Square for x^2, then a 2-pass DVE normalize with
    per-token scale/offset rows partition-broadcast by GpSimd.
  - LN's gamma is folded into the following weight matrix on the host;
    LN's beta contributes b@W which is applied as a per-feature bias at
    eviction (q,k) or through the V columns (softmax rows sum to 1).
  - Branch outputs are accumulated into the residual directly from PSUM with
    fused scalar_tensor_tensor evictions (one rounding per residual add).
  - Spatial attention runs on a frame-major copy of the normalized stream
    (strided per-frame normalize); temporal attention runs in token order
    with the S^T block-diagonal mask trick.
"""

import numpy as np
import ml_dtypes

import concourse.bass as bass
import concourse.mybir as mybir
import concourse.tile as tile
from concourse import bacc

F32 = mybir.dt.float32
BF16 = mybir.dt.bfloat16
AF = mybir.ActivationFunctionType
ALU = mybir.AluOpType
AX = mybir.AxisListType

D = 768
KT = 6
NH = 12
HD = 64
HID = 3072
B = 8
T = 8
HWn = 196
N = 1569
NPAD = 1664
NT = 1568
NF = 197
NS = T * NF
SCALE = HD ** -0.5
P = 128
EPS = 1e-5
INV_D = 1.0 / D

CH_T = [(0, 128), (128, 512), (640, 512), (1152, 416)]
CH_M = [(0, 128), (128, 512), (640, 512), (1152, 417)]
CH_QS = [(0, 1), (1, 1), (2, 2), (4, 2), (6, 2)]

BC_QKT = 0
BC_PRT = 12
BC_TFC = 18
BC_QKS = 24
BC_PRS = 36
BC_FC1 = 42
BC_FC2 = 66

VC_MASK = 0
VC_ID = 128
VC_VBT = 256
VC_VBS = 1024


def tiles_of(n, step=128):
    return [(i, min(step, n - i)) for i in range(0, n, step)]


def build_program(loop_n=0, sim_gelu=False):
    nc = bacc.Bacc("TRN2", target_bir_lowering=False, debug=False, num_devices=8)

    xbf = nc.dram_tensor("xbf", [NPAD, D], BF16, kind="ExternalInput").ap()
    wqkv_t_d = nc.dram_tensor("wqkv_t", [D, 3 * D], BF16, kind="ExternalInput").ap()
    wpr_t_d = nc.dram_tensor("wpr_t", [D, D], BF16, kind="ExternalInput").ap()
    wtfc_d = nc.dram_tensor("wtfc", [D, D], BF16, kind="ExternalInput").ap()
    wqkv_s_d = nc.dram_tensor("wqkv_s", [D, 3 * D], BF16, kind="ExternalInput").ap()
    wpr_s_d = nc.dram_tensor("wpr_s", [D, D], BF16, kind="ExternalInput").ap()
    w1_d = nc.dram_tensor("w1", [D, HID], BF16, kind="ExternalInput").ap()
    w2_d = nc.dram_tensor("w2", [HID, D], BF16, kind="ExternalInput").ap()
    biases_d = nc.dram_tensor("biases", [P, 72], F32, kind="ExternalInput").ap()
    vconst_d = nc.dram_tensor("vconst", [P, 1792], BF16, kind="ExternalInput").ap()
    out = nc.dram_tensor("out", [N, D], F32, kind="ExternalOutput").ap()

    from contextlib import nullcontext

    with tile.TileContext(nc) as tc:
      with tc.tile_pool(name="const", bufs=1) as const:
        eps_sb = const.tile([P, 1], F32, tag="eps")
        nc.vector.memset(eps_sb[:], EPS)
        ones_sb = const.tile([P, 8], BF16, tag="ones")
        nc.vector.memset(ones_sb[:], 1.0)
        oneD_sb = const.tile([P, 1], BF16, tag="oneD")
        nc.vector.memset(oneD_sb[:], INV_D)

        loop_cm = tc.For_i(0, loop_n, 1) if loop_n else nullcontext()
        with loop_cm:
          with tc.tile_pool(name="glob", bufs=1) as glob:
            xT = [glob.tile([P, NPAD], BF16, tag=f"xT{k}", name=f"xT{k}")
                  for k in range(KT)]
            for k in range(KT):
                nc.sync.dma_start(out=xT[k][:], in_=xbf[:, k * P:(k + 1) * P],
                                  transpose=True)
            Bt = glob.tile([P, 72], F32, tag="biases", name="biases")
            nc.sync.dma_start(out=Bt[:], in_=biases_d)
            vc = glob.tile([P, 1792], BF16, tag="vconst", name="vconst")
            nc.sync.dma_start(out=vc[:], in_=vconst_d)
            mask = vc[:, VC_MASK:VC_MASK + P]
            idb = vc[:, VC_ID:VC_ID + P]

            wq = [glob.tile([P, 3 * D], BF16, tag=f"wq{k}", name=f"wq{k}")
                  for k in range(KT)]
            wp = [glob.tile([P, D], BF16, tag=f"wp{k}", name=f"wp{k}")
                  for k in range(KT)]
            for k in range(KT):
                nc.sync.dma_start(out=wq[k][:], in_=wqkv_t_d[k * P:(k + 1) * P, :])
            for k in range(KT):
                nc.sync.dma_start(out=wp[k][:], in_=wpr_t_d[k * P:(k + 1) * P, :])

            cls_save = glob.tile([P, KT], F32, tag="cls", name="cls_save")
            for k in range(KT):
                nc.vector.tensor_copy(cls_save[:, k:k + 1], xT[k][:, 1:2])

            def ln_stats(pool, ps_pool, src_col0, c0, pc, a_bc, c_bc):
                """Per-token scale/offset rows for xT cols [src_col0+c0, +pc)."""
                # ones vector pre-scaled by 1/D: psA = mean, psB = E[x^2]
                psA = ps_pool.tile([P, 512], F32, tag="mm", name="sx")
                for k in range(KT):
                    src = xT[k][:, src_col0 + c0:src_col0 + c0 + pc]
                    nc.tensor.matmul(psA[0:1, :pc], oneD_sb[:, 0:1], src,
                                     start=(k == 0), stop=(k == KT - 1))
                psB = ps_pool.tile([P, 512], F32, tag="mm", name="sq")
                for k in range(KT):
                    src = xT[k][:, src_col0 + c0:src_col0 + c0 + pc]
                    sq = pool.tile([P, 512], BF16, tag="sqv", name="sqv", bufs=2)
                    nc.scalar.activation(sq[:, :pc], src, AF.Square)
                    nc.tensor.matmul(psB[0:1, :pc], oneD_sb[:, 0:1], sq[:, :pc],
                                     start=(k == 0), stop=(k == KT - 1))
                mu = pool.tile([1, 512], F32, tag="mu", name="mu", bufs=2)
                nc.vector.tensor_copy(mu[:, :pc], psA[0:1, :pc])
                r2 = pool.tile([1, 512], F32, tag="r2", name="r2", bufs=2)
                # var = E[x^2] - mu^2
                nc.vector.scalar_tensor_tensor(r2[:, :pc], mu[:, :pc], -1.0,
                                               mu[:, :pc], ALU.mult, ALU.mult)
                nc.vector.tensor_tensor(r2[:, :pc], psB[0:1, :pc], r2[:, :pc],
                                        ALU.add)
                # inv = exp(-0.5*ln(var+eps)); Ln and Exp live in the same ACT
                # table as Square/Identity (natural_log_exp_and_others)
                nc.scalar.activation(r2[:, :pc], r2[:, :pc], AF.Ln,
                                     bias=eps_sb[0:1])
                a_row = pool.tile([1, 512], BF16, tag="arow", name="arow", bufs=2)
                nc.scalar.activation(a_row[:, :pc], r2[:, :pc], AF.Exp, scale=-0.5)
                c_row = pool.tile([1, 512], BF16, tag="crow", name="crow", bufs=2)
                nc.vector.tensor_tensor(c_row[:, :pc], mu[:, :pc],
                                        a_row[:, :pc], ALU.mult)
                nc.gpsimd.partition_broadcast(a_bc[:, c0:c0 + pc], a_row[0:1, :pc])
                nc.gpsimd.partition_broadcast(c_bc[:, c0:c0 + pc], c_row[0:1, :pc])

            # =====================================================
            # Stage T
            # =====================================================
            with tc.tile_pool(name="t_sb", bufs=1) as sbT, \
                 tc.tile_pool(name="t_work", bufs=3) as work, \
                 tc.tile_pool(name="t_mm", bufs=3, space="PSUM") as pmm, \
                 tc.tile_pool(name="t_att", bufs=3, space="PSUM") as patt, \
                 tc.tile_pool(name="t_tr", bufs=2, space="PSUM") as ptr:

                wtfc = [sbT.tile([P, D], BF16, tag=f"wt{k}", name=f"wt{k}")
                        for k in range(KT)]
                for k in range(KT):
                    nc.sync.dma_start(out=wtfc[k][:], in_=wtfc_d[k * P:(k + 1) * P, :])

                aT = sbT.tile([P, NT], BF16, tag="aT", name="aT")
                cT = sbT.tile([P, NT], BF16, tag="cT", name="cT")
                lnT = [sbT.tile([P, NT], BF16, tag=f"ln{k}", name=f"lnT{k}")
                       for k in range(KT)]
                qkT = [sbT.tile([P, NT], BF16, tag=f"qk{j}", name=f"qkT{j}")
                       for j in range(12)]
                v_t = [sbT.tile([P, NH, HD + 1], BF16, tag=f"vt{g}", name=f"vt{g}")
                       for g in range(13)]
                oT = [sbT.tile([P, NT], BF16, tag=f"oT{k}", name=f"oTt{k}")
                      for k in range(KT)]

                for (c0, pc) in CH_T:
                    ln_stats(work, pmm, 1, c0, pc, aT, cT)
                    for k in range(KT):
                        tmp = work.tile([P, 512], BF16, tag="nt", name="nt", bufs=3)
                        nc.vector.tensor_tensor(tmp[:, :pc],
                                                xT[k][:, 1 + c0:1 + c0 + pc],
                                                aT[:, c0:c0 + pc], ALU.mult)
                        nc.vector.tensor_tensor(lnT[k][:, c0:c0 + pc], tmp[:, :pc],
                                                cT[:, c0:c0 + pc], ALU.subtract)
                    for mi in range(12):
                        ps = pmm.tile([P, 512], F32, tag="mm", name="mm")
                        for k in range(KT):
                            nc.tensor.matmul(ps[:, :pc],
                                             wq[k][:, mi * P:(mi + 1) * P],
                                             lnT[k][:, c0:c0 + pc],
                                             start=(k == 0), stop=(k == KT - 1))
                        nc.scalar.activation(qkT[mi][:, c0:c0 + pc], ps[:, :pc],
                                             AF.Identity, bias=Bt[:, BC_QKT + mi:BC_QKT + mi + 1])
                    for (g0, gp) in tiles_of(pc):
                        g = (c0 + g0) // P
                        t0 = c0 + g0
                        nc.vector.memset(v_t[g][:gp, :, HD:HD + 1], 1.0)
                        for half in range(2):
                            ps = pmm.tile([P, 512], F32, tag="mm", name="mmv")
                            for k in range(KT):
                                nc.tensor.matmul(
                                    ps[:gp, :384],
                                    lnT[k][:, t0:t0 + gp],
                                    wq[k][:, 2 * D + 384 * half:2 * D + 384 * (half + 1)],
                                    start=(k == 0), stop=(k == KT - 1))
                            nc.vector.scalar_tensor_tensor(
                                v_t[g][:gp, 6 * half:6 * (half + 1), 0:HD],
                                ps[:gp, :384].rearrange("p (a c) -> p a c", a=6),
                                1.0,
                                vc[0:gp, VC_VBT + 384 * half:VC_VBT + 384 * (half + 1)]
                                .rearrange("p (a c) -> p a c", a=6),
                                ALU.mult, ALU.add)
                    for (g0, gp) in tiles_of(pc):
                        g = (c0 + g0) // P
                        t0 = c0 + g0
                        o_tm = work.tile([P, D], BF16, tag="otm", name="otm", bufs=2)
                        for h in range(NH):
                            j, r0 = h // 2, HD * (h % 2)
                            att = patt.tile([P, P + HD + 1], F32, tag="att",
                                            name="att")
                            st = att[:, 0:P]
                            ov = att[:, P:P + HD + 1]
                            nc.tensor.matmul(st[:gp, :gp],
                                             qkT[6 + j][r0:r0 + HD, t0:t0 + gp],
                                             qkT[j][r0:r0 + HD, t0:t0 + gp],
                                             start=True, stop=True)
                            es = work.tile([P, P], BF16, tag="es", name="es", bufs=6)
                            nc.scalar.activation(es[:gp, :gp], st[:gp, :gp], AF.Exp,
                                                 scale=SCALE)
                            nc.vector.tensor_tensor(es[:gp, :gp], es[:gp, :gp],
                                                    mask[0:gp, 0:gp], ALU.mult)
                            nc.tensor.matmul(ov[:gp, :], es[:gp, :gp],
                                             v_t[g][:gp, h, :], start=True, stop=True)
                            rec = work.tile([P, 1], F32, tag="rec", name="rec", bufs=4)
                            nc.vector.reciprocal(rec[:gp], ov[:gp, HD:HD + 1])
                            nc.vector.tensor_scalar_mul(o_tm[:gp, HD * h:HD * (h + 1)],
                                                        ov[:gp, 0:HD], rec[:gp])
                        for k in range(KT):
                            ps = ptr.tile([P, P], BF16, tag="tr", name="tr")
                            nc.tensor.transpose(ps[:, :gp], o_tm[:gp, k * P:(k + 1) * P],
                                                idb[0:gp, 0:gp])
                            nc.vector.tensor_copy(oT[k][:, t0:t0 + gp],
                                                  ps[:, :gp])




                # proj -> pT (chunk-local) -> tfc -> residual accumulate
                for (c0, pc) in CH_T:
                    pTc = []
                    for mi in range(KT):
                        ps = pmm.tile([P, 512], F32, tag="mm", name="mmp")
                        for k in range(KT):
                            nc.tensor.matmul(ps[:, :pc],
                                             wp[k][:, mi * P:(mi + 1) * P],
                                             oT[k][:, c0:c0 + pc],
                                             start=(k == 0), stop=(k == KT - 1))
                        pT = work.tile([P, 512], BF16, tag=f"pT{mi}",
                                       name=f"pT{mi}", bufs=2)
                        nc.scalar.activation(pT[:, :pc], ps[:, :pc],
                                             AF.Identity, bias=Bt[:, BC_PRT + mi:BC_PRT + mi + 1])
                        pTc.append(pT)
                    for mi in range(KT):
                        ps = pmm.tile([P, 512], F32, tag="mm", name="mmt")
                        for k in range(KT):
                            nc.tensor.matmul(ps[:, :pc],
                                             wtfc[k][:, mi * P:(mi + 1) * P],
                                             pTc[k][:, :pc],
                                             start=(k == 0), stop=(k == KT - 1))
                        nc.vector.scalar_tensor_tensor(
                            xT[mi][:, 1 + c0:1 + c0 + pc], ps[:, :pc],
                            Bt[:, BC_TFC + mi:BC_TFC + mi + 1],
                            xT[mi][:, 1 + c0:1 + c0 + pc], ALU.add, ALU.add)

                # spatial weights go into the same tiles, after the last
                # temporal reads (program order guarantees correctness)
                for k in range(KT):
                    nc.sync.dma_start(out=wq[k][:], in_=wqkv_s_d[k * P:(k + 1) * P, :])
                for k in range(KT):
                    nc.sync.dma_start(out=wp[k][:], in_=wpr_s_d[k * P:(k + 1) * P, :])

            # =====================================================
            # Stage S
            # =====================================================
            with tc.tile_pool(name="s_sb", bufs=1) as sbS, \
                 tc.tile_pool(name="s_work", bufs=3) as work, \
                 tc.tile_pool(name="s_mm", bufs=4, space="PSUM") as pmm, \
                 tc.tile_pool(name="s_st", bufs=2, space="PSUM") as pst, \
                 tc.tile_pool(name="s_ov", bufs=2, space="PSUM") as pov:

                aS = sbS.tile([P, NT], BF16, tag="aS", name="aS")
                cS = sbS.tile([P, NT], BF16, tag="cS", name="cS")
                lnS = [sbS.tile([P, NS], BF16, tag=f"lnS{k}", name=f"lnS{k}")
                       for k in range(KT)]
                qkS = [sbS.tile([P, NS], BF16, tag=f"qkS{j}", name=f"qkS{j}")
                       for j in range(12)]
                oS = [sbS.tile([P, NS], BF16, tag=f"oS{k}", name=f"oSs{k}")
                      for k in range(KT)]

                lnStok = [sbS.tile([P, NT], BF16, tag=f"lnK{k}", name=f"lnStok{k}")
                          for k in range(KT)]
                for (c0, pc) in CH_T:
                    ln_stats(work, pmm, 1, c0, pc, aS, cS)
                    for k in range(KT):
                        tmp = work.tile([P, 512], BF16, tag="ns", name="ns", bufs=3)
                        nc.vector.tensor_tensor(tmp[:, :pc],
                                                xT[k][:, 1 + c0:1 + c0 + pc],
                                                aS[:, c0:c0 + pc], ALU.mult)
                        nc.vector.tensor_tensor(lnStok[k][:, c0:c0 + pc],
                                                tmp[:, :pc],
                                                cS[:, c0:c0 + pc], ALU.subtract)

                # cls token LN (from the saved original x[1])
                cls_bf = work.tile([P, KT], BF16, tag="clsbf", name="cls_bf")
                nc.vector.tensor_copy(cls_bf[:], cls_save[:])
                psc1 = pmm.tile([P, 512], F32, tag="mm", name="clsx")
                for k in range(KT):
                    nc.tensor.matmul(psc1[0:1, 0:1], ones_sb[:, 0:1],
                                     cls_bf[:, k:k + 1],
                                     start=(k == 0), stop=(k == KT - 1))
                sqc = work.tile([P, KT], BF16, tag="sqc", name="sqc")
                nc.scalar.activation(sqc[:], cls_save[:], AF.Square)
                psc2 = pmm.tile([P, 512], F32, tag="mm", name="clsq")
                for k in range(KT):
                    nc.tensor.matmul(psc2[0:1, 0:1], ones_sb[:, 0:1], sqc[:, k:k + 1],
                                     start=(k == 0), stop=(k == KT - 1))
                muc = work.tile([1, 2], F32, tag="muc", name="muc")
                nc.vector.tensor_scalar_mul(muc[:, 0:1], psc1[0:1, 0:1], INV_D)
                varc = work.tile([1, 1], F32, tag="varc", name="varc")
                nc.vector.scalar_tensor_tensor(varc[:], muc[:, 0:1], -1.0,
                                               muc[:, 0:1], ALU.mult, ALU.mult)
                nc.vector.scalar_tensor_tensor(varc[:], psc2[0:1, 0:1], INV_D,
                                               varc[:], ALU.mult, ALU.add)
                invc = work.tile([1, 1], F32, tag="invc", name="invc")
                nc.scalar.activation(invc[:], varc[:], AF.Ln, bias=eps_sb[0:1])
                nc.scalar.activation(invc[:], invc[:], AF.Exp, scale=-0.5)
                stc = work.tile([P, 2], F32, tag="stc", name="stc")
                nc.gpsimd.partition_broadcast(stc[:, 0:1], muc[:, 0:1])
                nc.gpsimd.partition_broadcast(stc[:, 1:2], invc[:, 0:1])
                lncls = work.tile([P, KT], F32, tag="lncls", name="lncls")
                nc.vector.tensor_scalar(lncls[:], cls_save[:], stc[:, 0:1],
                                        stc[:, 1:2], ALU.subtract, ALU.mult)
                for k in range(KT):
                    nc.vector.tensor_scalar_mul(
                        lnS[k].rearrange("p (t n) -> p t n", t=T)[:, :, 0:1],
                        ones_sb[:, 0:8].rearrange("p (t n) -> p t n", t=T),
                        lncls[:, k:k + 1])

                def fview(ap1568, f):
                    return (ap1568.rearrange("p (w t) -> p t w", t=T)
                            [:, f:f + 1, :].rearrange("p a w -> p (a w)"))

                # scatter token-order lnStok into frame-major lnS, split
                # across ACT and DVE
                for f in range(T):
                    for k in range(KT):
                        src = fview(lnStok[k][:], f)
                        dst = lnS[k][:, f * NF + 1:(f + 1) * NF]
                        if (f * KT + k) % 2 == 0:
                            nc.scalar.activation(dst, src, AF.Copy)
                        else:
                            nc.vector.tensor_copy(dst, src)

                for (f0, nf) in CH_QS:
                    c0, pc = f0 * NF, nf * NF
                    for mi in range(12):
                        ps = pmm.tile([P, 512], F32, tag="mm", name="mmqs")
                        for k in range(KT):
                            nc.tensor.matmul(ps[:, :pc],
                                             wq[k][:, mi * P:(mi + 1) * P],
                                             lnS[k][:, c0:c0 + pc],
                                             start=(k == 0), stop=(k == KT - 1))
                        nc.scalar.activation(qkS[mi][:, c0:c0 + pc], ps[:, :pc],
                                             AF.Identity, bias=Bt[:, BC_QKS + mi:BC_QKS + mi + 1])
                    v_s = {}
                    for f in range(f0, f0 + nf):
                        for i, (k0, pk) in enumerate(tiles_of(NF)):
                            v_s.setdefault(f, {})[i] = sbS.tile(
                                [P, NH, HD + 1], BF16,
                                tag=f"vs{(f % 2) * 2 + i}",
                                name=f"vs{(f % 2) * 2 + i}", bufs=2)
                            nc.vector.memset(v_s[f][i][:pk, :, HD:HD + 1], 1.0)
                            for half in range(2):
                                ps = pmm.tile([P, 512], F32, tag="mm", name="mmvs")
                                for k in range(KT):
                                    nc.tensor.matmul(
                                        ps[:pk, :384],
                                        lnS[k][:, f * NF + k0:f * NF + k0 + pk],
                                        wq[k][:, 2 * D + 384 * half:2 * D + 384 * (half + 1)],
                                        start=(k == 0), stop=(k == KT - 1))
                                nc.vector.scalar_tensor_tensor(
                                    v_s[f][i][:pk, 6 * half:6 * (half + 1), 0:HD],
                                    ps[:pk, :384].rearrange("p (a c) -> p a c", a=6),
                                    1.0,
                                    vc[0:pk, VC_VBS + 384 * half:VC_VBS + 384 * (half + 1)]
                                    .rearrange("p (a c) -> p a c", a=6),
                                    ALU.mult, ALU.add)
                    for f in range(f0, f0 + nf):
                        fc = f * NF
                        for h in range(NH):
                            j, r0 = h // 2, HD * (h % 2)
                            qs = qkS[j][r0:r0 + HD, fc:fc + NF]
                            es_list = []
                            for i, (k0, pk) in enumerate(tiles_of(NF)):
                                st = pst.tile([P, NF], F32, tag="st", name="stS")
                                nc.tensor.matmul(st[:pk, :NF],
                                                 qkS[6 + j][r0:r0 + HD,
                                                            fc + k0:fc + k0 + pk],
                                                 qs, start=True, stop=True)
                                es = work.tile([P, NF], BF16, tag="esS", name="esS",
                                               bufs=6)
                                nc.scalar.activation(es[:pk, :NF], st[:pk, :NF],
                                                     AF.Exp, scale=SCALE)
                                es_list.append((es, k0, pk))
                            ov = pov.tile([HD + 1, NF], F32, tag="ov", name="ovS")
                            for i, (es, k0, pk) in enumerate(es_list):
                                nc.tensor.matmul(ov[:, :NF], v_s[f][i][:pk, h, :],
                                                 es[:pk, :NF], start=(i == 0),
                                                 stop=(i == len(es_list) - 1))
                            rec = work.tile([1, NF], F32, tag="recS", name="recS",
                                            bufs=4)
                            nc.vector.reciprocal(rec[:1, :], ov[HD:HD + 1, :])
                            bc = work.tile([HD, NF], F32, tag="bcS", name="bcS",
                                           bufs=4)
                            nc.gpsimd.partition_broadcast(bc[:, :], rec[0:1, :])
                            nc.vector.tensor_tensor(oS[j][r0:r0 + HD, fc:fc + NF],
                                                    ov[0:HD, :NF], bc[:, :], ALU.mult)


                for (f0, nf) in CH_QS:
                    c0, pc = f0 * NF, nf * NF
                    for mi in range(KT):
                        ps = pmm.tile([P, 512], F32, tag="mm", name="mmps")
                        for k in range(KT):
                            nc.tensor.matmul(ps[:, :pc],
                                             wp[k][:, mi * P:(mi + 1) * P],
                                             oS[k][:, c0:c0 + pc],
                                             start=(k == 0), stop=(k == KT - 1))
                        for f in range(f0, f0 + nf):
                            off = (f - f0) * NF
                            nc.vector.scalar_tensor_tensor(
                                fview(xT[mi][:, 1:1 + NT], f),
                                ps[:, off + 1:off + NF],
                                Bt[:, BC_PRS + mi:BC_PRS + mi + 1],
                                fview(xT[mi][:, 1:1 + NT], f), ALU.add, ALU.add)

                # cls_out = proj(mean over frames of attention-out cls cols)
                oTc = work.tile([P, KT], BF16, tag="oTc", name="oTc")
                for k in range(KT):
                    red = work.tile([P, 1], F32, tag="redc", name="redc", bufs=2)
                    nc.vector.tensor_reduce(
                        red[:],
                        oS[k].rearrange("p (t n) -> p n t", t=T)[:, 0:1, :],
                        AX.X, ALU.add)
                    nc.vector.tensor_scalar_mul(oTc[:, k:k + 1], red[:], 1.0 / T)
                for mi in range(KT):
                    psc = pmm.tile([P, 512], F32, tag="mm", name="clsp")
                    for k in range(KT):
                        nc.tensor.matmul(psc[:, 0:1], wp[k][:, mi * P:(mi + 1) * P],
                                         oTc[:, k:k + 1],
                                         start=(k == 0), stop=(k == KT - 1))
                    nc.vector.scalar_tensor_tensor(
                        xT[mi][:, 0:1], psc[:, 0:1], Bt[:, BC_PRS + mi:BC_PRS + mi + 1],
                        cls_save[:, mi:mi + 1], ALU.add, ALU.add)

                # W2 into the soon-free wq/wp tiles (emitted after last reads)
                w2v = []
                for j in range(24):
                    if j < 18:
                        tgt = wq[j // 3][:, D * (j % 3):D * (j % 3 + 1)]
                    else:
                        tgt = wp[j - 18][:]
                    nc.sync.dma_start(out=tgt, in_=w2_d[j * P:(j + 1) * P, :])
                    w2v.append(tgt)

            # =====================================================
            # Stage M (MLP)
            # =====================================================
            with tc.tile_pool(name="m_sb", bufs=1) as sbM, \
                 tc.tile_pool(name="m_g", bufs=2) as gpool, \
                 tc.tile_pool(name="m_work", bufs=3) as work, \
                 tc.tile_pool(name="m_mm", bufs=5, space="PSUM") as pmm, \
                 tc.tile_pool(name="m_tr", bufs=3, space="PSUM") as ptr:

                aM = sbM.tile([P, N], BF16, tag="aM", name="aM")
                cM = sbM.tile([P, N], BF16, tag="cM", name="cM")
                lnM = [sbM.tile([P, N], BF16, tag=f"lnM{k}", name=f"lnM{k}")
                       for k in range(KT)]
                w1 = [sbM.tile([P, HID], BF16, tag=f"w1{k}", name=f"w1{k}")
                      for k in range(KT)]
                for k in range(KT):
                    nc.sync.dma_start(out=w1[k][:], in_=w1_d[k * P:(k + 1) * P, :])

                # all stats + normalizes first so the ACT table switches
                # ln_exp -> gelu exactly once per iteration
                for (c0, pc) in CH_M:
                    ln_stats(work, pmm, 0, c0, pc, aM, cM)
                    for k in range(KT):
                        tmp = work.tile([P, 512], BF16, tag="nt", name="ntm", bufs=3)
                        nc.vector.tensor_tensor(tmp[:, :pc], xT[k][:, c0:c0 + pc],
                                                aM[:, c0:c0 + pc], ALU.mult)
                        nc.vector.tensor_tensor(lnM[k][:, c0:c0 + pc], tmp[:, :pc],
                                                cM[:, c0:c0 + pc], ALU.subtract)
                for (c0, pc) in CH_M:
                    g1T = [gpool.tile([P, 512], BF16, tag=f"g1T{m}", name=f"g1T{m}")
                           for m in range(24)]
                    for m in range(24):
                        ps = pmm.tile([P, 512], F32, tag="mm", name="f1ps")
                        for k in range(KT):
                            nc.tensor.matmul(ps[:, :pc], w1[k][:, m * P:(m + 1) * P],
                                             lnM[k][:, c0:c0 + pc],
                                             start=(k == 0), stop=(k == KT - 1))
                        if sim_gelu:
                            hb = work.tile([P, 512], F32, tag="hb", name="hb", bufs=1)
                            nc.scalar.activation(hb[:, :pc], ps[:, :pc], AF.Identity,
                                                 bias=Bt[:, BC_FC1 + m:BC_FC1 + m + 1])
                            sg = work.tile([P, 512], F32, tag="sg", name="sg", bufs=1)
                            nc.scalar.activation(sg[:, :pc], hb[:, :pc], AF.Sigmoid,
                                                 scale=1.702)
                            nc.vector.tensor_tensor(g1T[m][:, :pc], hb[:, :pc],
                                                    sg[:, :pc], ALU.mult)
                        else:
                            nc.scalar.activation(g1T[m][:, :pc], ps[:, :pc], AF.Gelu,
                                                 bias=Bt[:, BC_FC1 + m:BC_FC1 + m + 1])
                    for mi in range(KT):
                        ps = pmm.tile([P, 512], F32, tag="mm", name="f2ps")
                        for k in range(24):
                            nc.tensor.matmul(ps[:, :pc], w2v[k][:, mi * P:(mi + 1) * P],
                                             g1T[k][:, :pc],
                                             start=(k == 0), stop=(k == 23))
                        nc.vector.scalar_tensor_tensor(
                            xT[mi][:, c0:c0 + pc], ps[:, :pc], Bt[:, BC_FC2 + mi:BC_FC2 + mi + 1],
                            xT[mi][:, c0:c0 + pc], ALU.add, ALU.add)
                    for (q0, pq) in tiles_of(pc):
                        t0 = c0 + q0
                        out_sb = work.tile([P, D], F32, tag="osb", name="osb", bufs=3)
                        for k in range(KT):
                            ps = ptr.tile([P, P], BF16, tag="tr", name="otr")
                            nc.tensor.transpose(ps[:pq, :], xT[k][:, t0:t0 + pq],
                                                idb[:, :])
                            nc.vector.tensor_copy(out_sb[:pq, k * P:(k + 1) * P],
                                                  ps[:pq, :])
                        nc.sync.dma_start(out=out[t0:t0 + pq, :], in_=out_sb[:pq])

    nc.compile()
    return nc


_CACHED = {}


def _get_program():
    if "nc" not in _CACHED:
        _CACHED["nc"] = build_program()
    return _CACHED["nc"]


def _host_prep(inputs):
    f32 = np.float32
    g = lambda k: np.asarray(inputs[k], f32)
    x = g("x")
    gt, bt = g("gt"), g("bt")
    g1, b1 = g("g1"), g("b1")
    g2, b2 = g("g2"), g("b2")
    Wqkv_t, Wproj_t, bproj_t = g("Wqkv_t"), g("Wproj_t"), g("bproj_t")
    Wqkv_s, Wproj_s, bproj_s = g("Wqkv_s"), g("Wproj_s"), g("bproj_s")
    Wtfc, btfc = g("Wtfc"), g("btfc")
    W1, b1m = g("W1"), g("b1m")
    W2, b2m = g("W2"), g("b2m")

    bf = ml_dtypes.bfloat16
    wqkv_t = np.ascontiguousarray((gt[:, None] * Wqkv_t).astype(bf))
    wqkv_s = np.ascontiguousarray((g1[:, None] * Wqkv_s).astype(bf))
    w1 = np.ascontiguousarray((g2[:, None] * W1).astype(bf))
    qkvb_t = bt @ Wqkv_t
    qkvb_s = b1 @ Wqkv_s
    b1m_f = b2 @ W1 + b1m

    def cols(vec, n):
        return np.asarray(vec, f32).reshape(n, P).T

    biases = np.concatenate([
        cols(qkvb_t[:2 * D], 12), cols(bproj_t, 6), cols(btfc, 6),
        cols(qkvb_s[:2 * D], 12), cols(bproj_s, 6),
        cols(b1m_f, 24), cols(b2m, 6)], axis=1).astype(f32)

    mask = np.kron(np.eye(16, dtype=f32), np.ones((8, 8), f32))
    ident = np.eye(P, dtype=f32)
    vb_t = np.tile(qkvb_t[2 * D:], (P, 1))
    vb_s = np.tile(qkvb_s[2 * D:], (P, 1))
    vconst = np.concatenate([mask, ident, vb_t, vb_s], axis=1).astype(bf)

    base = {
        "wqkv_t": wqkv_t, "wpr_t": np.ascontiguousarray(Wproj_t.astype(bf)),
        "wtfc": np.ascontiguousarray(Wtfc.astype(bf)),
        "wqkv_s": wqkv_s, "wpr_s": np.ascontiguousarray(Wproj_s.astype(bf)),
        "w1": w1, "w2": np.ascontiguousarray(W2.astype(bf)),
        "biases": np.ascontiguousarray(biases),
        "vconst": np.ascontiguousarray(vconst),
    }
    maps = []
    for i in range(B):
        xb = np.zeros((NPAD, D), bf)
        xb[:N] = x[i].astype(bf)
        maps.append(dict(base, xbf=np.ascontiguousarray(xb)))
    return maps


def make_in_maps(inputs):
    return _host_prep(inputs)


def kernel(**inputs):
    nc = _get_program()
    in_maps = make_in_maps(inputs)
    core_ids = list(range(8))
    from concourse.bass_utils import run_bass_kernel_spmd
    res = run_bass_kernel_spmd(nc, in_maps, core_ids)
    return np.stack([res.results[i]["out"] for i in core_ids], axis=0)


if __name__ == "__main__":
    build_program()
    print("built ok")
